# revision 3
# baseline (speedup 1.0000x reference)
"""BitLinear (ternary group-quantized linear) Trainium2 Bass kernel.

Computes: w_q = groupwise_ternary_quantize(weight, group=128 along in_features)
          out = x @ w_q.T + bias
for x (4, 2048, 4096) f32, weight (16384, 4096) f32, bias (16384,) f32.

Sharding (tensor-parallel, per the row-sharding strategy):
  - weight rows (out_features) and bias sharded 8 ways: 2048 rows/core
  - x replicated to all 8 cores
  - each core computes its (8192, 2048) output slice; host concatenates.

Per-core kernel (SPMD, identical program, different input data):
  Phase C: cast x f32 -> bf16 (scalar engine), staged to DRAM scratch per
           256-row block so the matmul phase can consume blocks as they land.
  Phase Q: quantize the 2048x4096 weight shard on-chip:
           group sums via reduce(|w|), scale = max(mean,eps), ternary
           compare/subtract, scale multiply (vector engine, f32 math so the
           threshold decisions match the f32 reference bit-for-bit), output
           bf16, then PE-transpose into an SBUF-resident K-major cache
           [128, 32, 512] x 4 strips (the matmul's moving operand).
  Phase M: composable_matmul_tile_kernel: stationary = xT bf16 tiles loaded
           straight from the bf16 scratch with XBAR DMA-transpose; moving =
           the SBUF-resident quantized weight cache; psum f32; bias added
           during psum->sbuf eviction; f32 written out.
"""

import os
from contextlib import ExitStack
from dataclasses import replace

import numpy as np

import concourse.bass as bass
import concourse.mybir as mybir
import concourse.tile as tile
from concourse import bacc
from concourse.bass import ds, ts
from concourse.bass_utils import run_bass_kernel_spmd
from concourse.kernels.tile_matmul import (
    ShapeInfo,
    composable_matmul_tile_kernel,
    dma_from_dram_kxm,
    dma_to_dram_mxn,
)
from concourse.masks import make_identity

F32 = mybir.dt.float32
BF16 = mybir.dt.bfloat16
P = 128

N_CORES = 8
M_FULL = 8192          # 4*2048 tokens
K = 4096               # in_features
N_OUT_FULL = 16384     # out_features
N = N_OUT_FULL // N_CORES  # 2048 out rows per core
KG = K // P            # 32 contraction groups of 128 (also the quant groups)
MB = 256               # m batch (token block) size in phase M
N_STRIP = 512          # kxn cache strip width (= matmul N_TILE)
QK = 1024              # k-chunk for the quant/cast temps (SBUF pressure)


def build_kernel(tc: tile.TileContext, ctx: ExitStack, m_tokens: int):
    nc = tc.nc
    nb_m = m_tokens // MB
    n_rt = N // P          # 16 weight row-tiles
    n_strips = N // N_STRIP  # 4

    x_ap = nc.dram_tensor("x", [m_tokens, K], F32, kind="ExternalInput").ap()
    w_ap = nc.dram_tensor("w", [N, K], F32, kind="ExternalInput").ap()
    biasb_ap = nc.dram_tensor("biasb", [P, N], F32, kind="ExternalInput").ap()
    out_ap = nc.dram_tensor("out", [m_tokens, N], F32, kind="ExternalOutput").ap()

    const = ctx.enter_context(tc.tile_pool(name="const", bufs=1))
    cache_pool = ctx.enter_context(tc.tile_pool(name="kxncache", bufs=1))
    dram = ctx.enter_context(tc.tile_pool(name="dram", bufs=1, space="DRAM"))

    identity = const.tile([P, P], BF16, tag="ident")
    make_identity(nc, identity[:])

    # K-major quantized-weight cache, SBUF resident: strip s holds out-rows
    # [512*s, 512*(s+1)) for all k: [p = k % 128, gk = k // 128, row]
    cache_strips = [
        cache_pool.tile([P, KG, N_STRIP], BF16, tag=f"kxnc{s}", name=f"kxnc{s}")
        for s in range(n_strips)
    ]

    # bf16 x scratch, one DRAM tile per 256-row block (dep granularity)
    xb_tiles = [
        dram.tile([MB, K], BF16, tag=f"xb{b}", name=f"xb{b}") for b in range(nb_m)
    ]

    # ---------------- Phases C (cast x) and Q (quantize w), interleaved ----
    with (
        tc.tile_pool(name="qc", bufs=2) as qc,
        tc.tile_pool(name="qsmall", bufs=2) as qsmall,
        tc.tile_pool(name="qpsum", bufs=2, space="PSUM") as qpsum,
    ):
        for i in range(max(nb_m, n_rt)):
            if i < nb_m:
                # cast rows [256i, 256i+256) of x to bf16
                for sub in range(MB // P):
                    r0 = sub * P
                    for h in range(K // QK):
                        c0 = h * QK
                        xf = qc.tile([P, QK], F32, tag="xf")
                        nc.sync.dma_start(
                            xf[:], x_ap[ds(i * MB + r0, P), ds(c0, QK)]
                        )
                        xbf = qc.tile([P, QK], BF16, tag="xbf")
                        nc.scalar.activation(
                            xbf[:], xf[:], mybir.ActivationFunctionType.Copy
                        )
                        nc.sync.dma_start(
                            xb_tiles[i][ds(r0, P), ds(c0, QK)], xbf[:]
                        )
            if i < n_rt:
                # quantize weight rows [128i, 128i+128)
                rt = i
                strip = rt // (N_STRIP // P)
                col = (rt % (N_STRIP // P)) * P
                for h in range(K // QK):
                    gq = QK // P  # groups in this chunk
                    wf = qc.tile([P, gq, P], F32, tag="wf")
                    nc.sync.dma_start(
                        wf[:], w_ap[ds(rt * P, P), ds(h * QK, QK)]
                    )
                    gsum = qsmall.tile([P, gq, 1], F32, tag="gsum")
                    nc.vector.tensor_reduce(
                        gsum[:], wf[:], axis=mybir.AxisListType.X,
                        op=mybir.AluOpType.add, apply_absolute_value=True,
                    )
                    scale = qsmall.tile([P, gq, 1], F32, tag="scale")
                    nc.vector.tensor_scalar(
                        scale[:], gsum[:], 1.0 / P, 1e-8,
                        op0=mybir.AluOpType.mult, op1=mybir.AluOpType.max,
                    )
                    thr = qsmall.tile([P, gq, 1], F32, tag="thr")
                    nc.vector.tensor_scalar(
                        thr[:], scale[:], 0.5, None, op0=mybir.AluOpType.mult
                    )
                    nthr = qsmall.tile([P, gq, 1], F32, tag="nthr")
                    nc.vector.tensor_scalar(
                        nthr[:], scale[:], -0.5, None, op0=mybir.AluOpType.mult
                    )
                    pos = qc.tile([P, gq, P], F32, tag="pos")
                    _, thr_b = bass.broadcast_tensor_aps(pos[:], thr[:])
                    nc.vector.tensor_tensor(
                        pos[:], wf[:], thr_b, op=mybir.AluOpType.is_gt
                    )
                    neg = qc.tile([P, gq, P], F32, tag="neg")
                    _, nthr_b = bass.broadcast_tensor_aps(neg[:], nthr[:])
                    nc.vector.tensor_tensor(
                        neg[:], wf[:], nthr_b, op=mybir.AluOpType.is_lt
                    )
                    nc.vector.tensor_tensor(
                        pos[:], pos[:], neg[:], op=mybir.AluOpType.subtract
                    )
                    wqb = qc.tile([P, gq, P], BF16, tag="wqb")
                    _, scale_b = bass.broadcast_tensor_aps(pos[:], scale[:])
                    nc.vector.tensor_tensor(
                        wqb[:], pos[:], scale_b, op=mybir.AluOpType.mult
                    )
                    # transpose each [128 rows, 128 k] group block into the cache
                    for g in range(gq):
                        gk = h * gq + g
                        pt = qpsum.tile([P, P], BF16, tag="qps")
                        nc.tensor.transpose(pt[:], wqb[:, g, :], identity[:])
                        nc.any.tensor_copy(
                            out=cache_strips[strip][:, gk, ds(col, P)], in_=pt[:]
                        )

    # ---------------- Phase M: out = xT.T @ wqT + bias --------------------
    biasb_sb = const.tile([P, N], F32, tag="biasb")
    nc.sync.dma_start(biasb_sb[:], biasb_ap)

    kxm_pool = ctx.enter_context(tc.tile_pool(name="kxm", bufs=10))

    producers = []
    for b in range(nb_m):
        prod, _shape = dma_from_dram_kxm(
            kxm_pool, xb_tiles[b][:], transpose_ap=True
        )
        producers.append(prod)

    def kxm_producer(nc_, md):
        return producers[md.m_batch_idx](nc_, replace(md, m_batch_idx=0))

    def kxn_producer(nc_, md):
        assert md.n_tile == N_STRIP and md.n_batch_idx == 0
        return cache_strips[md.n_tile_idx][:, ts(md.k_tile_idx, md.k_subtiles), :]

    consumers = [
        dma_to_dram_mxn(out_ap[ds(b * MB, MB), :]) for b in range(nb_m)
    ]

    def mxn_consumer(nc_, sbuf_tile, md):
        consumers[md.m_batch_idx](nc_, sbuf_tile, replace(md, m_batch_idx=0))

    def bias_reducer(nc_, psum, sbuf, md):
        off = md.n_tile_idx * md.n_tile + md.n_subtile_idx * md.n_subtile
        nc_.vector.tensor_tensor(
            out=sbuf[:, 0, :],
            in0=psum,
            in1=biasb_sb[:, ds(off, md.n_subtile)],
            op=mybir.AluOpType.add,
        )

    kxm_shape = ShapeInfo(pdims=((P, KG),), fdims=(MB,) * nb_m)
    kxn_shape = ShapeInfo(pdims=((P, KG),), fdims=(N,))

    composable_matmul_tile_kernel(
        tc=tc,
        kxm_shape=kxm_shape,
        kxn_shape=kxn_shape,
        output_type=F32,
        kxm_producer=kxm_producer,
        kxn_producer=kxn_producer,
        mxn_consumer=mxn_consumer,
        mxn_subtile_reducer=bias_reducer,
        MATMUL_FREE_DIM=512,
        MAX_TILE_SIZE=512,
        MAX_K_TILE_SIZE=512,
        cache_tiles=True,
        psum_n_bufs=2,
    )


def build_program(m_tokens: int = M_FULL):
    nc = bacc.Bacc(
        "TRN2",
        target_bir_lowering=False,
        debug=False,
        enable_asserts=False,
        num_devices=N_CORES,
    )
    with tile.TileContext(nc) as tc, ExitStack() as ctx:
        build_kernel(tc, ctx, m_tokens)
    nc.compile()
    return nc


_program_cache = {}


def _get_program(m_tokens: int):
    if m_tokens not in _program_cache:
        _program_cache[m_tokens] = build_program(m_tokens)
    return _program_cache[m_tokens]


def make_in_maps(x: np.ndarray, weight: np.ndarray, bias: np.ndarray):
    """Shard the full inputs for the 8 cores: replicate x, split w/bias rows."""
    xf = np.ascontiguousarray(x.reshape(-1, K).astype(np.float32, copy=False))
    in_maps = []
    for c in range(N_CORES):
        wsh = np.ascontiguousarray(weight[c * N:(c + 1) * N])
        bsh = bias[c * N:(c + 1) * N]
        biasb = np.ascontiguousarray(
            np.broadcast_to(bsh[None, :], (P, N)).astype(np.float32, copy=False)
        )
        in_maps.append({"x": xf, "w": wsh, "biasb": biasb})
    return in_maps


def kernel(x: np.ndarray, weight: np.ndarray, bias: np.ndarray):
    nc = _get_program(x.shape[0] * x.shape[1])
    in_maps = make_in_maps(x, weight, bias)
    res = run_bass_kernel_spmd(nc, in_maps, core_ids=list(range(N_CORES)))
    out = np.concatenate([res.results[c]["out"] for c in range(N_CORES)], axis=1)
    kernel.last_results = res
    return out.reshape(x.shape[0], x.shape[1], N_OUT_FULL).astype(np.float32)


def time_kernel(x: np.ndarray, weight: np.ndarray, bias: np.ndarray, iters: int = 5):
    """Time the on-device NEFF execution with device-resident inputs.

    Mirrors bass2jax.run_bass_via_pjrt's multi-core path, but stages the
    concatenated inputs on the devices once and times repeated executions
    (fresh donated output buffers each iter, staged outside the timed
    region). Returns (best_seconds, out_full ndarray).
    """
    import time

    import jax
    from jax.experimental.shard_map import shard_map
    from jax.sharding import Mesh, PartitionSpec

    from concourse import bass2jax
    from concourse.bass2jax import _bass_exec_p, install_neuronx_cc_hook

    install_neuronx_cc_hook()
    nc = _get_program(x.shape[0] * x.shape[1])
    in_maps = make_in_maps(x, weight, bias)

    partition_name = (
        nc.partition_id_tensor.name if nc.partition_id_tensor else None
    )
    in_names, out_names, out_avals, zero_outs = [], [], [], []
    for alloc in nc.m.functions[0].allocations:
        if not isinstance(alloc, mybir.MemoryLocationSet):
            continue
        name = alloc.memorylocations[0].name
        if alloc.kind == "ExternalInput":
            if name != partition_name:
                in_names.append(name)
        elif alloc.kind == "ExternalOutput":
            shape = tuple(alloc.tensor_shape)
            dtype = mybir.dt.np(alloc.dtype)
            out_avals.append(jax.core.ShapedArray(shape, dtype))
            out_names.append(name)
            zero_outs.append(np.zeros(shape, dtype))
    n_params = len(in_names)
    n_outs = len(out_avals)
    all_in_names = list(in_names) + list(out_names)
    if partition_name is not None:
        all_in_names.append(partition_name)
    donate = tuple(range(n_params, n_params + n_outs))

    def _body(*args):
        operands = list(args)
        if partition_name is not None:
            operands.append(bass2jax.partition_id_tensor())
        outs = _bass_exec_p.bind(
            *operands,
            out_avals=tuple(out_avals),
            in_names=tuple(all_in_names),
            out_names=tuple(out_names),
            lowering_input_output_aliases=(),
            sim_require_finite=True,
            sim_require_nnan=True,
            nc=nc,
        )
        return tuple(outs)

    devices = jax.devices()[:N_CORES]
    mesh = Mesh(np.asarray(devices), ("core",))
    in_specs = (PartitionSpec("core"),) * (n_params + n_outs)
    out_specs = (PartitionSpec("core"),) * n_outs
    sharded = jax.jit(
        shard_map(_body, mesh=mesh, in_specs=in_specs, out_specs=out_specs,
                  check_rep=False),
        donate_argnums=donate,
        keep_unused=True,
    )
    from jax.sharding import NamedSharding

    shard = NamedSharding(mesh, PartitionSpec("core"))
    concat_in = [
        jax.device_put(
            np.concatenate([np.asarray(in_maps[c][nm]) for c in range(N_CORES)], axis=0),
            shard,
        )
        for nm in in_names
    ]
    best = None
    out_arrs = None
    for _ in range(iters):
        zeros_dev = [
            jax.device_put(np.zeros((N_CORES * z.shape[0], *z.shape[1:]), z.dtype), shard)
            for z in zero_outs
        ]
        jax.block_until_ready(zeros_dev)
        jax.block_until_ready(concat_in)
        t0 = time.perf_counter()
        out_arrs = sharded(*concat_in, *zeros_dev)
        jax.block_until_ready(out_arrs)
        dt = time.perf_counter() - t0
        print(f"  iter: {dt * 1e3:.3f} ms")
        if best is None or dt < best:
            best = dt
    i_out = out_names.index("out")
    out = np.asarray(out_arrs[i_out]).reshape(N_CORES, x.shape[0] * x.shape[1], N)
    out_full = np.concatenate([out[c] for c in range(N_CORES)], axis=1)
    return best, out_full.reshape(x.shape[0], x.shape[1], N_OUT_FULL)
